# revision 32
# baseline (speedup 1.0000x reference)
"""Distributed Bass kernel for nn_AttentionLayer (B=2, S=2048, D=1024, H=16).

Sharding: tensor-parallel over heads. Core c owns heads {2c, 2c+1} (128 of the
1024 hidden dims). Each core:
  - projects q/k/v for its heads over all 4096 tokens (inputs fed pre-transposed
    as X^T so the contraction dim lands on SBUF partitions),
  - computes transposed scores scoreT[s,t] = k_h . q_h with the two heads packed
    into the PE array as K=64 row-tiles writing adjacent PSUM banks, adds the
    shared attn bias (b=0: identity-stationary matmul into PSUM on the PE;
    b=1: tensor_tensor add on the otherwise-idle DVE), exp on ScalarE over the
    combined [128,1024] tile,
  - PV matmul with V (natural [s,dk] layout) as the stationary operand,
    augmented with a ones column so softmax denominators fall out of row 64,
  - AllToAll switches head-sharding -> token-sharding (each core ends up with
    all heads for its 512-token slice), normalizes, and applies the output
    projection for its token slice.
Host side reassembles (out, cache_k, cache_v) from per-core slices.
"""

import sys

import numpy as np

for _p in ("/opt/trn_rl_repo",):
    if _p not in sys.path:
        sys.path.insert(0, _p)

import ml_dtypes

BF = ml_dtypes.bfloat16

B, S, D, H = 2, 2048, 1024, 16
DK = D // H            # 64
NCORES = 8
T = B * S              # 4096
OC = D // NCORES       # 128 hidden dims per core (2 heads)
TSL = T // NCORES      # 512 token slice per core after AllToAll

_CACHE = {}


def _build_nc():
    import concourse.bass as bass
    import concourse.mybir as mybir
    import concourse.tile as tile
    from concourse import bacc

    f32 = mybir.dt.float32
    bf16 = mybir.dt.bfloat16
    AF = mybir.ActivationFunctionType

    nc = bacc.Bacc(
        "TRN2",
        target_bir_lowering=False,
        debug=False,
        num_devices=NCORES,
    )

    # ---- kernel I/O ----
    xq = nc.dram_tensor("xq_t", [D, T], bf16, kind="ExternalInput")
    xk = nc.dram_tensor("xk_t", [D, T], bf16, kind="ExternalInput")
    xv = nc.dram_tensor("xv_t", [D, T], bf16, kind="ExternalInput")
    ebias_t = nc.dram_tensor("ebias_t", [B, S, S], bf16, kind="ExternalInput")
    wq_t = nc.dram_tensor("wq_t", [D, OC], bf16, kind="ExternalInput")
    wk_t = nc.dram_tensor("wk_t", [D, OC], bf16, kind="ExternalInput")
    wv_t = nc.dram_tensor("wv_t", [D, OC], bf16, kind="ExternalInput")
    wo_t = nc.dram_tensor("wo_t", [D, D], bf16, kind="ExternalInput")
    bq_d = nc.dram_tensor("bq_c", [OC, 1], f32, kind="ExternalInput")
    bk_d = nc.dram_tensor("bk_c", [OC, 1], f32, kind="ExternalInput")
    bv_d = nc.dram_tensor("bv_c", [OC, 1], f32, kind="ExternalInput")
    bo_d = nc.dram_tensor("bo_f", [D, 1], f32, kind="ExternalInput")
    sel_d = nc.dram_tensor("sel", [H, D], f32, kind="ExternalInput")
    ident_d = nc.dram_tensor("ident", [128, 128], bf16, kind="ExternalInput")

    kt_out = nc.dram_tensor("kt_out", [OC, T], f32, kind="ExternalOutput")
    vt_out = nc.dram_tensor("vt_out", [OC, T], f32, kind="ExternalOutput")
    out_t = nc.dram_tensor("out_t", [D, TSL], f32, kind="ExternalOutput")

    rg = [list(range(NCORES))]

    with tile.TileContext(nc) as tc:
        with tc.tile_pool(name="persist", bufs=1) as pp, \
             tc.tile_pool(name="dramp", bufs=1, space="DRAM") as dramp:
            ident = pp.tile([128, 128], bf16)
            nc.sync.dma_start(ident[:], ident_d.ap())

            wq_sb = pp.tile([128, 8, OC], bf16)
            wk_sb = pp.tile([128, 8, OC], bf16)
            wv_sb = pp.tile([128, 8, OC], bf16)
            nc.sync.dma_start(wq_sb[:], wq_t.ap().rearrange("(c p) m -> p c m", p=128))
            nc.sync.dma_start(wk_sb[:], wk_t.ap().rearrange("(c p) m -> p c m", p=128))
            nc.sync.dma_start(wv_sb[:], wv_t.ap().rearrange("(c p) m -> p c m", p=128))
            wo_sb = pp.tile([128, 8, D], bf16)
            nc.sync.dma_start(wo_sb[:], wo_t.ap().rearrange("(c p) m -> p c m", p=128))
            sel_sb = pp.tile([H, D], f32)
            nc.sync.dma_start(sel_sb[:], sel_d.ap())
            bq_sb = pp.tile([OC, 1], f32)
            bk_sb = pp.tile([OC, 1], f32)
            nc.sync.dma_start(bq_sb[:], bq_d.ap())
            nc.sync.dma_start(bk_sb[:], bk_d.ap())
            bv_sb = pp.tile([OC, 1], f32)
            nc.sync.dma_start(bv_sb[:], bv_d.ap())
            bo_sb = pp.tile([128, 8], f32)
            nc.sync.dma_start(bo_sb[:], bo_d.ap().rearrange("(c p) o -> p (c o)", p=128))
            # persistent activations
            qt_sb = pp.tile([OC, T], bf16)       # qT for this core's 2 heads
            kt_sb = pp.tile([OC, T], bf16)       # kT
            vt_sb = pp.tile([OC, T], bf16)       # vT (transposed to v_sb below)
            # v in natural [s, dk] layout; per 128-token chunk the free axis is
            # [vA(64) | ones | vB(64) | ones] so head slices 0:65 / 65:130 give
            # the ones-augmented PV stationary directly.
            v_sb = pp.tile([128, T // 128, 130], bf16)
            nc.vector.memset(v_sb[:, :, 64:65], 1.0)
            nc.vector.memset(v_sb[:, :, 129:130], 1.0)

            # collective bounce buffers (DRAM)
            a2a_in = dramp.tile([NCORES, OC + 4, TSL], bf16)
            a2a_out = dramp.tile([NCORES, OC + 4, TSL], bf16)

            # ================= phase 1: projections =================
            # d-chunk outer; one 1MB X^T chunk DMA feeds 8 token-chunk matmuls
            # (k, v) or accumulates into 8 PSUM banks (k/q: one per t-chunk;
            # v: 4 128-token tiles packed per bank).
            with tc.tile_pool(name="proj_ps", bufs=8, space="PSUM") as prps, \
                 tc.tile_pool(name="proj_sb", bufs=3) as prsb, \
                 tc.tile_pool(name="proj_ev", bufs=3) as prev:
                # --- K projection (kT layout: [o, t]) ---
                ps_k = [prps.tile([128, 512], f32, tag="ps", name=f"ps_k{i}") for i in range(8)]
                for dc in range(8):
                    xt = prsb.tile([128, T], bf16, tag="xch")
                    xr = xk.ap()[dc * 128:(dc + 1) * 128, :]
                    for pc in range(8):
                        eng = (nc.sync, nc.scalar, nc.gpsimd)[pc % 3]
                        eng.dma_start(xt[:, pc * 512:(pc + 1) * 512],
                                      xr[:, pc * 512:(pc + 1) * 512])
                    for tcg in range(8):
                        nc.tensor.matmul(ps_k[tcg][:], wk_sb[:, dc, :],
                                         xt[:, tcg * 512:(tcg + 1) * 512],
                                         start=(dc == 0), stop=(dc == 7))
                for tcg in range(8):
                    kf = prev.tile([128, 512], f32, tag="kf32")
                    nc.vector.tensor_scalar_add(kf[:], ps_k[tcg][:], bk_sb[:, 0:1])
                    nc.sync.dma_start(kt_out.ap()[:, tcg * 512:(tcg + 1) * 512], kf[:])
                    nc.vector.tensor_copy(kt_sb[:, tcg * 512:(tcg + 1) * 512], kf[:])

                # --- V projection (vT layout like K; transposed afterwards) ---
                ps_v = [prps.tile([128, 512], f32, tag="ps", name=f"ps_v{i}") for i in range(8)]
                for dc in range(8):
                    xt = prsb.tile([128, T], bf16, tag="xch")
                    xr = xv.ap()[dc * 128:(dc + 1) * 128, :]
                    for pc in range(8):
                        eng = (nc.sync, nc.scalar, nc.gpsimd)[pc % 3]
                        eng.dma_start(xt[:, pc * 512:(pc + 1) * 512],
                                      xr[:, pc * 512:(pc + 1) * 512])
                    for tcg in range(8):
                        nc.tensor.matmul(ps_v[tcg][:], wv_sb[:, dc, :],
                                         xt[:, tcg * 512:(tcg + 1) * 512],
                                         start=(dc == 0), stop=(dc == 7))
                for tcg in range(8):
                    vf = prev.tile([128, 512], f32, tag="vf32")
                    nc.vector.tensor_scalar_add(vf[:], ps_v[tcg][:], bv_sb[:, 0:1])
                    nc.sync.dma_start(vt_out.ap()[:, tcg * 512:(tcg + 1) * 512], vf[:])
                    nc.vector.tensor_copy(vt_sb[:, tcg * 512:(tcg + 1) * 512], vf[:])

                # --- Q projection (qT layout; 1/sqrt(dk) folded in host-side) ---
                ps_q = [prps.tile([128, 512], f32, tag="ps", name=f"ps_q{i}") for i in range(8)]
                for dc in range(8):
                    xt = prsb.tile([128, T], bf16, tag="xch")
                    xr = xq.ap()[dc * 128:(dc + 1) * 128, :]
                    for pc in range(8):
                        eng = (nc.sync, nc.scalar, nc.gpsimd)[pc % 3]
                        eng.dma_start(xt[:, pc * 512:(pc + 1) * 512],
                                      xr[:, pc * 512:(pc + 1) * 512])
                    for tcg in range(8):
                        nc.tensor.matmul(ps_q[tcg][:], wq_sb[:, dc, :],
                                         xt[:, tcg * 512:(tcg + 1) * 512],
                                         start=(dc == 0), stop=(dc == 7))
                for tcg in range(8):
                    nc.vector.tensor_scalar_add(
                        qt_sb[:, tcg * 512:(tcg + 1) * 512], ps_q[tcg][:],
                        bq_sb[:, 0:1])

            # --- transpose vT -> v natural [s, dk] with ones-augmented layout ---
            with tc.tile_pool(name="tr_ps", bufs=4, space="PSUM") as trps:
                for stg in range(T // 128):
                    ps_t = trps.tile([128, 128], bf16, tag="ps_t")
                    nc.tensor.transpose(
                        ps_t[:], vt_sb[:, stg * 128:(stg + 1) * 128], ident[:])
                    nc.vector.tensor_copy(v_sb[:, stg, 0:64], ps_t[:, 0:64])
                    nc.vector.tensor_copy(v_sb[:, stg, 65:129], ps_t[:, 64:128])

            # ================= phase 2: attention =================
            # scores for both heads land in one [128,1024] (2-bank) PSUM tile:
            # head A in [:,0:512], head B in [:,512:1024] (row-packed K=64
            # matmuls into adjacent banks run concurrently), one FD=1024 exp.
            with tc.tile_pool(name="sc_ps", bufs=3, space="PSUM") as scps, \
                 tc.tile_pool(name="pv_ps", bufs=1, space="PSUM") as pvps, \
                 tc.tile_pool(name="at_sb", bufs=3) as atsb:
                for b in range(B):
                    for tcc in range(4):
                        j = b * 4 + tcc            # destination core / t slice
                        tg = b * S + tcc * 512     # global token offset
                        pv_A = pvps.tile([65, 512], f32, tag="pv_A")
                        pv_B = pvps.tile([65, 512], f32, tag="pv_B")
                        for st in range(S // 128):
                            stg = b * (S // 128) + st
                            bias_tile = atsb.tile([128, 512], bf16, tag="bias")
                            nc.gpsimd.dma_start(
                                bias_tile[:],
                                ebias_t.ap()[b, st * 128:(st + 1) * 128,
                                             tcc * 512:(tcc + 1) * 512])
                            ps = scps.tile([128, 1024], f32, tag="ps_sc")
                            # transposed scores, two heads as K=64 row tiles
                            nc.tensor.matmul(
                                ps[:, 0:512],
                                kt_sb[0:64, stg * 128:(stg + 1) * 128],
                                qt_sb[0:64, tg:tg + 512],
                                start=True, stop=True)
                            nc.tensor.matmul(
                                ps[:, 512:1024],
                                kt_sb[64:128, stg * 128:(stg + 1) * 128],
                                qt_sb[64:128, tg:tg + 512],
                                start=True, stop=True)
                            # p = exp(score) * exp(bias): exp on ACT, then a
                            # cheap bf16 2x-mode multiply on DVE
                            pr = atsb.tile([128, 1024], bf16, tag="pr")
                            nc.scalar.activation(pr[:], ps[:], AF.Exp)
                            pt = atsb.tile([128, 1024], bf16, tag="pt")
                            nc.vector.tensor_tensor(
                                pt[:, 0:512], pr[:, 0:512], bias_tile[:],
                                mybir.AluOpType.mult)
                            nc.vector.tensor_tensor(
                                pt[:, 512:1024], pr[:, 512:1024], bias_tile[:],
                                mybir.AluOpType.mult)
                            nc.tensor.matmul(pv_A[:], v_sb[:, stg, 0:65],
                                             pt[:, 0:512],
                                             start=(st == 0), stop=(st == 15))
                            nc.tensor.matmul(pv_B[:], v_sb[:, stg, 65:130],
                                             pt[:, 512:1024],
                                             start=(st == 0), stop=(st == 15))
                        at_A = atsb.tile([64, 512], bf16, tag="at_A")
                        at_B = atsb.tile([64, 512], bf16, tag="at_B")
                        nc.vector.tensor_copy(at_A[:], pv_A[0:64, :])
                        nc.vector.tensor_copy(at_B[:], pv_B[0:64, :])
                        nc.sync.dma_start(a2a_in[j, 0:64, :], at_A[:])
                        nc.sync.dma_start(a2a_in[j, 64:128, :], at_B[:])
                        dn = atsb.tile([65, 2, 512], f32, tag="dn")
                        nc.vector.tensor_copy(dn[64:65, 0, :], pv_A[64:65, :])
                        nc.vector.tensor_copy(dn[64:65, 1, :], pv_B[64:65, :])
                        nc.sync.dma_start(a2a_in[j, OC:OC + 4, :],
                                          dn[64:65, :, :].bitcast(bf16))

                # keep the PE busy (and the HAM clock warm) across the
                # AllToAll window with a chained throwaway accumulation
                warm = pvps.tile([65, 512], f32, tag="pv_A", name="warm")
                for wi in range(72):
                    nc.tensor.matmul(warm[:], v_sb[:, 31, 0:65], pt[:, 0:512],
                                     start=(wi == 0), stop=(wi == 71))

            # ================= phase 3: collectives =================
            nc.gpsimd.collective_compute(
                "AllToAll", mybir.AluOpType.bypass, replica_groups=rg,
                ins=[a2a_in[:].opt()], outs=[a2a_out[:].opt()])

            # ================= phase 4: normalize + output proj =================
            with tc.tile_pool(name="po_ps", bufs=8, space="PSUM") as pops, \
                 tc.tile_pool(name="po_sb", bufs=2) as posb:
                attn_rv = pp.tile([128, 8, TSL], bf16)
                nc.sync.dma_start(attn_rv[:],
                                  a2a_out[:, 0:OC, :].rearrange("c p f -> p c f"))
                den_rv = posb.tile([H, TSL], f32, tag="den_rv")
                den_sp = den_rv[:].rearrange("(c u) f -> c u f", u=2)
                for u in range(2):
                    nc.sync.dma_start(
                        den_sp[:, u, :],
                        a2a_out[:, OC + 2 * u:OC + 2 * u + 2, :].bitcast(
                            f32).rearrange("c t f -> c (t f)"))
                rcp = posb.tile([H, TSL], f32, tag="rcp")
                nc.vector.reciprocal(rcp[:], den_rv[:])
                attn_n = pp.tile([128, 8, TSL], bf16)
                rgs = []
                for oc in range(8):
                    ps_rg = pops.tile([128, 512], f32, tag="ps_rg",
                                      name=f"ps_rg{oc}")
                    nc.tensor.matmul(
                        ps_rg[:], sel_sb[:, oc * 128:(oc + 1) * 128], rcp[:],
                        start=True, stop=True)
                    rgs.append(ps_rg)
                for oc in range(8):
                    nc.vector.tensor_tensor(attn_n[:, oc, :], attn_rv[:, oc, :],
                                            rgs[oc][:], mybir.AluOpType.mult)
                for do in range(8):
                    ps_o = pops.tile([128, 512], f32, tag="ps_rg", name=f"ps_o{do}")
                    for oc in range(8):
                        nc.tensor.matmul(
                            ps_o[:], wo_sb[:, oc, do * 128:(do + 1) * 128],
                            attn_n[:, oc, :],
                            start=(oc == 0), stop=(oc == 7))
                    of = posb.tile([128, 512], f32, tag="of")
                    nc.vector.tensor_scalar_add(of[:], ps_o[:], bo_sb[:, do:do + 1])
                    nc.sync.dma_start(out_t.ap()[do * 128:(do + 1) * 128, :], of[:])

    return nc


def _get_nc():
    if "nc" not in _CACHE:
        nc = _build_nc()
        if not nc.is_finalized():
            nc.finalize()
        _CACHE["nc"] = nc
    return _CACHE["nc"]


def _prepare_in_maps(queries, keys, values, attn_bias, Wq, bq, Wk, bk, Wv, bv,
                     Wo, bo):
    f32 = np.float32
    xq_t = np.ascontiguousarray(
        np.asarray(queries, f32).reshape(T, D).T).astype(BF)
    xk_t = np.ascontiguousarray(
        np.asarray(keys, f32).reshape(T, D).T).astype(BF)
    xv_t = np.ascontiguousarray(
        np.asarray(values, f32).reshape(T, D).T).astype(BF)
    ebias_t = np.ascontiguousarray(
        np.exp(np.transpose(np.asarray(attn_bias, f32)[:, 0], (0, 2, 1)))).astype(BF)

    Wq = np.asarray(Wq, f32); Wk = np.asarray(Wk, f32)
    Wv = np.asarray(Wv, f32); Wo = np.asarray(Wo, f32)
    bq = np.asarray(bq, f32); bk = np.asarray(bk, f32)
    bv = np.asarray(bv, f32); bo = np.asarray(bo, f32)

    scale = 1.0 / np.sqrt(np.float32(DK))
    wo_t = np.ascontiguousarray(Wo.T).astype(BF)
    bo_f = np.ascontiguousarray(bo.reshape(D, 1))
    sel = np.zeros((H, D), np.float32)
    for o in range(D):
        sel[o // DK, o] = 1.0

    in_maps = []
    for c in range(NCORES):
        sl = slice(c * OC, (c + 1) * OC)
        in_maps.append({
            "xq_t": xq_t, "xk_t": xk_t, "xv_t": xv_t, "ebias_t": ebias_t,
            "wq_t": np.ascontiguousarray((Wq[sl] * scale).T).astype(BF),
            "wk_t": np.ascontiguousarray(Wk[sl].T).astype(BF),
            "wv_t": np.ascontiguousarray(Wv[sl].T).astype(BF),
            "wo_t": wo_t,
            "bq_c": np.ascontiguousarray((bq[sl] * scale).reshape(OC, 1)),
            "bk_c": np.ascontiguousarray(bk[sl].reshape(OC, 1)),
            "bv_c": np.ascontiguousarray(bv[sl].reshape(OC, 1)),
            "bo_f": bo_f,
            "sel": sel,
            "ident": np.eye(128, dtype=np.float32).astype(BF),
        })
    return in_maps


def _run(in_maps, trace=False):
    from concourse.bass_utils import run_bass_kernel_spmd

    nc = _get_nc()
    return run_bass_kernel_spmd(nc, in_maps, core_ids=list(range(NCORES)),
                                trace=trace)


def _assemble(results):
    out_full = np.empty((T, D), np.float32)
    k_full = np.empty((T, D), np.float32)
    v_full = np.empty((T, D), np.float32)
    for c in range(NCORES):
        r = results[c]
        k_full[:, c * OC:(c + 1) * OC] = r["kt_out"].T
        v_full[:, c * OC:(c + 1) * OC] = r["vt_out"].T
        out_full[c * TSL:(c + 1) * TSL, :] = r["out_t"].T
    return (out_full.reshape(B, S, D), k_full.reshape(B, S, D),
            v_full.reshape(B, S, D))


def kernel(**inputs):
    in_maps = _prepare_in_maps(**inputs)
    res = _run(in_maps, trace=False)
    return _assemble(res.results)


# revision 33
# speedup vs baseline: 1.0401x; 1.0401x over previous
"""Distributed Bass kernel for nn_AttentionLayer (B=2, S=2048, D=1024, H=16).

Sharding: tensor-parallel over heads. Core c owns heads {2c, 2c+1} (128 of the
1024 hidden dims). Each core:
  - projects q/k/v for its heads over all 4096 tokens (inputs fed pre-transposed
    as X^T so the contraction dim lands on SBUF partitions),
  - computes transposed scores scoreT[s,t] = k_h . q_h with the two heads packed
    into the PE array as K=64 row-tiles writing adjacent PSUM banks, adds the
    shared attn bias (b=0: identity-stationary matmul into PSUM on the PE;
    b=1: tensor_tensor add on the otherwise-idle DVE), exp on ScalarE over the
    combined [128,1024] tile,
  - PV matmul with V (natural [s,dk] layout) as the stationary operand,
    augmented with a ones column so softmax denominators fall out of row 64,
  - AllToAll switches head-sharding -> token-sharding (each core ends up with
    all heads for its 512-token slice), normalizes, and applies the output
    projection for its token slice.
Host side reassembles (out, cache_k, cache_v) from per-core slices.
"""

import sys

import numpy as np

for _p in ("/opt/trn_rl_repo",):
    if _p not in sys.path:
        sys.path.insert(0, _p)

import ml_dtypes

BF = ml_dtypes.bfloat16

B, S, D, H = 2, 2048, 1024, 16
DK = D // H            # 64
NCORES = 8
T = B * S              # 4096
OC = D // NCORES       # 128 hidden dims per core (2 heads)
TSL = T // NCORES      # 512 token slice per core after AllToAll

_CACHE = {}


def _build_nc():
    import concourse.bass as bass
    import concourse.mybir as mybir
    import concourse.tile as tile
    from concourse import bacc

    f32 = mybir.dt.float32
    bf16 = mybir.dt.bfloat16
    AF = mybir.ActivationFunctionType

    nc = bacc.Bacc(
        "TRN2",
        target_bir_lowering=False,
        debug=False,
        num_devices=NCORES,
    )

    # ---- kernel I/O ----
    xq = nc.dram_tensor("xq_t", [D, T], bf16, kind="ExternalInput")
    xk = nc.dram_tensor("xk_t", [D, T], bf16, kind="ExternalInput")
    xv = nc.dram_tensor("xv_t", [D, T], bf16, kind="ExternalInput")
    ebias_t = nc.dram_tensor("ebias_t", [B, S, S], bf16, kind="ExternalInput")
    wq_t = nc.dram_tensor("wq_t", [D, OC], bf16, kind="ExternalInput")
    wk_t = nc.dram_tensor("wk_t", [D, OC], bf16, kind="ExternalInput")
    wv_t = nc.dram_tensor("wv_t", [D, OC], bf16, kind="ExternalInput")
    wo_t = nc.dram_tensor("wo_t", [D, D], bf16, kind="ExternalInput")
    bq_d = nc.dram_tensor("bq_c", [OC, 1], f32, kind="ExternalInput")
    bk_d = nc.dram_tensor("bk_c", [OC, 1], f32, kind="ExternalInput")
    bv_d = nc.dram_tensor("bv_c", [OC, 1], f32, kind="ExternalInput")
    bo_d = nc.dram_tensor("bo_f", [D, 1], f32, kind="ExternalInput")
    sel_d = nc.dram_tensor("sel", [H, D], f32, kind="ExternalInput")
    ident_d = nc.dram_tensor("ident", [128, 128], bf16, kind="ExternalInput")

    kt_out = nc.dram_tensor("kt_out", [OC, T], f32, kind="ExternalOutput")
    vt_out = nc.dram_tensor("vt_out", [OC, T], f32, kind="ExternalOutput")
    out_t = nc.dram_tensor("out_t", [D, TSL], f32, kind="ExternalOutput")

    rg = [list(range(NCORES))]

    with tile.TileContext(nc) as tc:
        with tc.tile_pool(name="persist", bufs=1) as pp, \
             tc.tile_pool(name="dramp", bufs=1, space="DRAM") as dramp:
            ident = pp.tile([128, 128], bf16)
            nc.sync.dma_start(ident[:], ident_d.ap())

            wq_sb = pp.tile([128, 8, OC], bf16)
            wk_sb = pp.tile([128, 8, OC], bf16)
            wv_sb = pp.tile([128, 8, OC], bf16)
            nc.sync.dma_start(wq_sb[:], wq_t.ap().rearrange("(c p) m -> p c m", p=128))
            nc.sync.dma_start(wk_sb[:], wk_t.ap().rearrange("(c p) m -> p c m", p=128))
            nc.sync.dma_start(wv_sb[:], wv_t.ap().rearrange("(c p) m -> p c m", p=128))
            wo_sb = pp.tile([128, 8, D], bf16)
            nc.sync.dma_start(wo_sb[:], wo_t.ap().rearrange("(c p) m -> p c m", p=128))
            sel_sb = pp.tile([H, D], f32)
            nc.sync.dma_start(sel_sb[:], sel_d.ap())
            bq_sb = pp.tile([OC, 1], f32)
            bk_sb = pp.tile([OC, 1], f32)
            nc.sync.dma_start(bq_sb[:], bq_d.ap())
            nc.sync.dma_start(bk_sb[:], bk_d.ap())
            bv_sb = pp.tile([OC, 1], f32)
            nc.sync.dma_start(bv_sb[:], bv_d.ap())
            bo_sb = pp.tile([128, 8], f32)
            nc.sync.dma_start(bo_sb[:], bo_d.ap().rearrange("(c p) o -> p (c o)", p=128))
            # persistent activations
            qt_sb = pp.tile([OC, T], bf16)       # qT for this core's 2 heads
            kt_sb = pp.tile([OC, T], bf16)       # kT
            vt_sb = pp.tile([OC, T], bf16)       # vT (transposed to v_sb below)
            # v in natural [s, dk] layout; per 128-token chunk the free axis is
            # [vA(64) | ones | vB(64) | ones] so head slices 0:65 / 65:130 give
            # the ones-augmented PV stationary directly.
            v_sb = pp.tile([128, T // 128, 130], bf16)
            nc.vector.memset(v_sb[:, :, 64:65], 1.0)
            nc.vector.memset(v_sb[:, :, 129:130], 1.0)

            # collective bounce buffers (DRAM)
            a2a_in = dramp.tile([NCORES, OC + 4, TSL], bf16)
            a2a_out = dramp.tile([NCORES, OC + 4, TSL], bf16)

            # ================= phase 1: projections =================
            # d-chunk outer; one 1MB X^T chunk DMA feeds 8 token-chunk matmuls
            # (k, v) or accumulates into 8 PSUM banks (k/q: one per t-chunk;
            # v: 4 128-token tiles packed per bank).
            with tc.tile_pool(name="proj_ps", bufs=8, space="PSUM") as prps, \
                 tc.tile_pool(name="proj_sb", bufs=3) as prsb, \
                 tc.tile_pool(name="proj_ev", bufs=3) as prev:
                # --- K projection (kT layout: [o, t]) ---
                ps_k = [prps.tile([128, 512], f32, tag="ps", name=f"ps_k{i}") for i in range(8)]
                for dc in range(8):
                    xt = prsb.tile([128, T], bf16, tag="xch")
                    xr = xk.ap()[dc * 128:(dc + 1) * 128, :]
                    for pc in range(4):
                        eng = (nc.sync, nc.scalar)[pc % 2]
                        eng.dma_start(xt[:, pc * 1024:(pc + 1) * 1024],
                                      xr[:, pc * 1024:(pc + 1) * 1024])
                    for tcg in range(8):
                        nc.tensor.matmul(ps_k[tcg][:], wk_sb[:, dc, :],
                                         xt[:, tcg * 512:(tcg + 1) * 512],
                                         start=(dc == 0), stop=(dc == 7))
                for tcg in range(8):
                    kf = prev.tile([128, 512], f32, tag="kf32")
                    nc.vector.tensor_scalar_add(kf[:], ps_k[tcg][:], bk_sb[:, 0:1])
                    nc.sync.dma_start(kt_out.ap()[:, tcg * 512:(tcg + 1) * 512], kf[:])
                    nc.vector.tensor_copy(kt_sb[:, tcg * 512:(tcg + 1) * 512], kf[:])

                # --- V projection (vT layout like K; transposed afterwards) ---
                ps_v = [prps.tile([128, 512], f32, tag="ps", name=f"ps_v{i}") for i in range(8)]
                for dc in range(8):
                    xt = prsb.tile([128, T], bf16, tag="xch")
                    xr = xv.ap()[dc * 128:(dc + 1) * 128, :]
                    for pc in range(4):
                        eng = (nc.sync, nc.scalar)[pc % 2]
                        eng.dma_start(xt[:, pc * 1024:(pc + 1) * 1024],
                                      xr[:, pc * 1024:(pc + 1) * 1024])
                    for tcg in range(8):
                        nc.tensor.matmul(ps_v[tcg][:], wv_sb[:, dc, :],
                                         xt[:, tcg * 512:(tcg + 1) * 512],
                                         start=(dc == 0), stop=(dc == 7))
                for tcg in range(8):
                    vf = prev.tile([128, 512], f32, tag="vf32")
                    nc.vector.tensor_scalar_add(vf[:], ps_v[tcg][:], bv_sb[:, 0:1])
                    nc.sync.dma_start(vt_out.ap()[:, tcg * 512:(tcg + 1) * 512], vf[:])
                    nc.vector.tensor_copy(vt_sb[:, tcg * 512:(tcg + 1) * 512], vf[:])

                # --- Q projection (qT layout; 1/sqrt(dk) folded in host-side) ---
                ps_q = [prps.tile([128, 512], f32, tag="ps", name=f"ps_q{i}") for i in range(8)]
                for dc in range(8):
                    xt = prsb.tile([128, T], bf16, tag="xch")
                    xr = xq.ap()[dc * 128:(dc + 1) * 128, :]
                    for pc in range(4):
                        eng = (nc.sync, nc.scalar)[pc % 2]
                        eng.dma_start(xt[:, pc * 1024:(pc + 1) * 1024],
                                      xr[:, pc * 1024:(pc + 1) * 1024])
                    for tcg in range(8):
                        nc.tensor.matmul(ps_q[tcg][:], wq_sb[:, dc, :],
                                         xt[:, tcg * 512:(tcg + 1) * 512],
                                         start=(dc == 0), stop=(dc == 7))
                for tcg in range(8):
                    nc.vector.tensor_scalar_add(
                        qt_sb[:, tcg * 512:(tcg + 1) * 512], ps_q[tcg][:],
                        bq_sb[:, 0:1])

            # --- transpose vT -> v natural [s, dk] with ones-augmented layout ---
            with tc.tile_pool(name="tr_ps", bufs=4, space="PSUM") as trps:
                for stg in range(T // 128):
                    ps_t = trps.tile([128, 128], bf16, tag="ps_t")
                    nc.tensor.transpose(
                        ps_t[:], vt_sb[:, stg * 128:(stg + 1) * 128], ident[:])
                    nc.vector.tensor_copy(v_sb[:, stg, 0:64], ps_t[:, 0:64])
                    nc.vector.tensor_copy(v_sb[:, stg, 65:129], ps_t[:, 64:128])

            # ================= phase 2: attention =================
            # scores for both heads land in one [128,1024] (2-bank) PSUM tile:
            # head A in [:,0:512], head B in [:,512:1024] (row-packed K=64
            # matmuls into adjacent banks run concurrently), one FD=1024 exp.
            with tc.tile_pool(name="sc_ps", bufs=3, space="PSUM") as scps, \
                 tc.tile_pool(name="pv_ps", bufs=1, space="PSUM") as pvps, \
                 tc.tile_pool(name="at_sb", bufs=3) as atsb:
                for b in range(B):
                    for tcc in range(4):
                        j = b * 4 + tcc            # destination core / t slice
                        tg = b * S + tcc * 512     # global token offset
                        pv_A = pvps.tile([65, 512], f32, tag="pv_A")
                        pv_B = pvps.tile([65, 512], f32, tag="pv_B")
                        for st in range(S // 128):
                            stg = b * (S // 128) + st
                            bias_tile = atsb.tile([128, 512], bf16, tag="bias")
                            nc.gpsimd.dma_start(
                                bias_tile[:],
                                ebias_t.ap()[b, st * 128:(st + 1) * 128,
                                             tcc * 512:(tcc + 1) * 512])
                            ps = scps.tile([128, 1024], f32, tag="ps_sc")
                            # transposed scores, two heads as K=64 row tiles
                            nc.tensor.matmul(
                                ps[:, 0:512],
                                kt_sb[0:64, stg * 128:(stg + 1) * 128],
                                qt_sb[0:64, tg:tg + 512],
                                start=True, stop=True)
                            nc.tensor.matmul(
                                ps[:, 512:1024],
                                kt_sb[64:128, stg * 128:(stg + 1) * 128],
                                qt_sb[64:128, tg:tg + 512],
                                start=True, stop=True)
                            # p = exp(score) * exp(bias): exp on ACT, then a
                            # cheap bf16 2x-mode multiply on DVE
                            pr = atsb.tile([128, 1024], bf16, tag="pr")
                            nc.scalar.activation(pr[:], ps[:], AF.Exp)
                            pt = atsb.tile([128, 1024], bf16, tag="pt")
                            nc.vector.tensor_tensor(
                                pt[:, 0:512], pr[:, 0:512], bias_tile[:],
                                mybir.AluOpType.mult)
                            nc.vector.tensor_tensor(
                                pt[:, 512:1024], pr[:, 512:1024], bias_tile[:],
                                mybir.AluOpType.mult)
                            nc.tensor.matmul(pv_A[:], v_sb[:, stg, 0:65],
                                             pt[:, 0:512],
                                             start=(st == 0), stop=(st == 15))
                            nc.tensor.matmul(pv_B[:], v_sb[:, stg, 65:130],
                                             pt[:, 512:1024],
                                             start=(st == 0), stop=(st == 15))
                        at_A = atsb.tile([64, 512], bf16, tag="at_A")
                        at_B = atsb.tile([64, 512], bf16, tag="at_B")
                        nc.vector.tensor_copy(at_A[:], pv_A[0:64, :])
                        nc.vector.tensor_copy(at_B[:], pv_B[0:64, :])
                        nc.sync.dma_start(a2a_in[j, 0:64, :], at_A[:])
                        nc.sync.dma_start(a2a_in[j, 64:128, :], at_B[:])
                        dn = atsb.tile([65, 2, 512], f32, tag="dn")
                        nc.vector.tensor_copy(dn[64:65, 0, :], pv_A[64:65, :])
                        nc.vector.tensor_copy(dn[64:65, 1, :], pv_B[64:65, :])
                        nc.sync.dma_start(a2a_in[j, OC:OC + 4, :],
                                          dn[64:65, :, :].bitcast(bf16))

                # keep the PE busy (and the HAM clock warm) across the
                # AllToAll window with a chained throwaway accumulation
                warm = pvps.tile([65, 512], f32, tag="pv_A", name="warm")
                for wi in range(72):
                    nc.tensor.matmul(warm[:], v_sb[:, 31, 0:65], pt[:, 0:512],
                                     start=(wi == 0), stop=(wi == 71))

            # ================= phase 3: collectives =================
            nc.gpsimd.collective_compute(
                "AllToAll", mybir.AluOpType.bypass, replica_groups=rg,
                ins=[a2a_in[:].opt()], outs=[a2a_out[:].opt()])

            # ================= phase 4: normalize + output proj =================
            with tc.tile_pool(name="po_ps", bufs=8, space="PSUM") as pops, \
                 tc.tile_pool(name="po_sb", bufs=2) as posb:
                attn_rv = pp.tile([128, 8, TSL], bf16)
                nc.sync.dma_start(attn_rv[:],
                                  a2a_out[:, 0:OC, :].rearrange("c p f -> p c f"))
                den_rv = posb.tile([H, TSL], f32, tag="den_rv")
                den_sp = den_rv[:].rearrange("(c u) f -> c u f", u=2)
                for u in range(2):
                    nc.sync.dma_start(
                        den_sp[:, u, :],
                        a2a_out[:, OC + 2 * u:OC + 2 * u + 2, :].bitcast(
                            f32).rearrange("c t f -> c (t f)"))
                rcp = posb.tile([H, TSL], f32, tag="rcp")
                nc.vector.reciprocal(rcp[:], den_rv[:])
                attn_n = pp.tile([128, 8, TSL], bf16)
                rgs = []
                for oc in range(8):
                    ps_rg = pops.tile([128, 512], f32, tag="ps_rg",
                                      name=f"ps_rg{oc}")
                    nc.tensor.matmul(
                        ps_rg[:], sel_sb[:, oc * 128:(oc + 1) * 128], rcp[:],
                        start=True, stop=True)
                    rgs.append(ps_rg)
                for oc in range(8):
                    nc.vector.tensor_tensor(attn_n[:, oc, :], attn_rv[:, oc, :],
                                            rgs[oc][:], mybir.AluOpType.mult)
                for do in range(8):
                    ps_o = pops.tile([128, 512], f32, tag="ps_rg", name=f"ps_o{do}")
                    for oc in range(8):
                        nc.tensor.matmul(
                            ps_o[:], wo_sb[:, oc, do * 128:(do + 1) * 128],
                            attn_n[:, oc, :],
                            start=(oc == 0), stop=(oc == 7))
                    of = posb.tile([128, 512], f32, tag="of")
                    nc.vector.tensor_scalar_add(of[:], ps_o[:], bo_sb[:, do:do + 1])
                    nc.sync.dma_start(out_t.ap()[do * 128:(do + 1) * 128, :], of[:])

    return nc


def _get_nc():
    if "nc" not in _CACHE:
        nc = _build_nc()
        if not nc.is_finalized():
            nc.finalize()
        _CACHE["nc"] = nc
    return _CACHE["nc"]


def _prepare_in_maps(queries, keys, values, attn_bias, Wq, bq, Wk, bk, Wv, bv,
                     Wo, bo):
    f32 = np.float32
    xq_t = np.ascontiguousarray(
        np.asarray(queries, f32).reshape(T, D).T).astype(BF)
    xk_t = np.ascontiguousarray(
        np.asarray(keys, f32).reshape(T, D).T).astype(BF)
    xv_t = np.ascontiguousarray(
        np.asarray(values, f32).reshape(T, D).T).astype(BF)
    ebias_t = np.ascontiguousarray(
        np.exp(np.transpose(np.asarray(attn_bias, f32)[:, 0], (0, 2, 1)))).astype(BF)

    Wq = np.asarray(Wq, f32); Wk = np.asarray(Wk, f32)
    Wv = np.asarray(Wv, f32); Wo = np.asarray(Wo, f32)
    bq = np.asarray(bq, f32); bk = np.asarray(bk, f32)
    bv = np.asarray(bv, f32); bo = np.asarray(bo, f32)

    scale = 1.0 / np.sqrt(np.float32(DK))
    wo_t = np.ascontiguousarray(Wo.T).astype(BF)
    bo_f = np.ascontiguousarray(bo.reshape(D, 1))
    sel = np.zeros((H, D), np.float32)
    for o in range(D):
        sel[o // DK, o] = 1.0

    in_maps = []
    for c in range(NCORES):
        sl = slice(c * OC, (c + 1) * OC)
        in_maps.append({
            "xq_t": xq_t, "xk_t": xk_t, "xv_t": xv_t, "ebias_t": ebias_t,
            "wq_t": np.ascontiguousarray((Wq[sl] * scale).T).astype(BF),
            "wk_t": np.ascontiguousarray(Wk[sl].T).astype(BF),
            "wv_t": np.ascontiguousarray(Wv[sl].T).astype(BF),
            "wo_t": wo_t,
            "bq_c": np.ascontiguousarray((bq[sl] * scale).reshape(OC, 1)),
            "bk_c": np.ascontiguousarray(bk[sl].reshape(OC, 1)),
            "bv_c": np.ascontiguousarray(bv[sl].reshape(OC, 1)),
            "bo_f": bo_f,
            "sel": sel,
            "ident": np.eye(128, dtype=np.float32).astype(BF),
        })
    return in_maps


def _run(in_maps, trace=False):
    from concourse.bass_utils import run_bass_kernel_spmd

    nc = _get_nc()
    return run_bass_kernel_spmd(nc, in_maps, core_ids=list(range(NCORES)),
                                trace=trace)


def _assemble(results):
    out_full = np.empty((T, D), np.float32)
    k_full = np.empty((T, D), np.float32)
    v_full = np.empty((T, D), np.float32)
    for c in range(NCORES):
        r = results[c]
        k_full[:, c * OC:(c + 1) * OC] = r["kt_out"].T
        v_full[:, c * OC:(c + 1) * OC] = r["vt_out"].T
        out_full[c * TSL:(c + 1) * TSL, :] = r["out_t"].T
    return (out_full.reshape(B, S, D), k_full.reshape(B, S, D),
            v_full.reshape(B, S, D))


def kernel(**inputs):
    in_maps = _prepare_in_maps(**inputs)
    res = _run(in_maps, trace=False)
    return _assemble(res.results)
